# revision 16
# baseline (speedup 1.0000x reference)
"""Trainium2 Bass kernel for a 3-branch GCN layer (sum of three GCNConvs).

Math: out[b,t] = sum_k A_k @ (x[b,t] @ W_k) + b_k, with A_k the normalized
adjacency (self loops) of the k-th tiny 25-node graph shared across (B,T).

Instead of the dense [1600x1600] kron operator (one big GEMM, ~395k PE
row-cycles/core), factor into two chained PE stages with NO on-chip
transposes (host pre-transposes x, which is free):

  stage W:  Y[btn, (k,c)] = X[btn, c'] @ [W1|W2|W3]      (K=64, F=192)
  stage A:  out[btn, c]   = sum_k kron(I5, A_k^T) @ Y_k  (K=125, F=64 x3)

Tiles are 125 rows = 5 (b,t) groups x 25 nodes, so the graph contraction
is a fixed 125x125 block-diagonal stationary per branch (~184k PE
row-cycles/core total). PSUM is managed as one 8-bank ring; each bank
holds one tile's Y accumulation region and its out region (psum
accumulation state is bank-granular, so never two accumulation groups
per bank). Y is cast fp32->fp16 by batched pair-copies spread over
DVE/ACT/GPSIMD; out is DMA'd directly from PSUM as fp32.

Data-parallel over batch: 8 batches (2400 bt rows) per core x 8 cores.
Bias is added on the host (typically zero; np.any fast-path).
"""

import sys

import numpy as np

if "/opt/trn_rl_repo" not in sys.path:
    sys.path.insert(0, "/opt/trn_rl_repo")

B, T, NNODES, C = 64, 300, 25, 64
N_CORES = 8
BT_LOC = (B // N_CORES) * T          # 2400 (b,t) rows per core
ROWS_LOC = BT_LOC * NNODES           # 60000 btn rows per core
TILE = 125                           # 5 bt-groups x 25 nodes
NTILES = ROWS_LOC // TILE            # 480
NGRP = NTILES // 4                   # 120 groups of 4 tiles
NCHUNK = 8                           # x input DMA chunks
HALF = NTILES // 2                   # tiles per partition-half (240)
CHW = ROWS_LOC // NCHUNK             # x chunk width in elements (7500)
BANKC = 512                          # fp32 elems per psum bank partition-row
OOFF = 256                           # out region offset within a bank

_PROGRAM_CACHE = {}
# extra kwargs for run_bass_kernel_spmd (test harness sets trace=True here)
_RUN_KW = {}


def _dense_adj(edge_index_k: np.ndarray) -> np.ndarray:
    """PyG GCNConv normalized dense adjacency A[dst, src] (float64)."""
    row = edge_index_k[0].astype(np.int64)
    col = edge_index_k[1].astype(np.int64)
    loop = np.arange(NNODES, dtype=np.int64)
    row = np.concatenate([row, loop])
    col = np.concatenate([col, loop])
    deg = np.zeros(NNODES, dtype=np.float64)
    np.add.at(deg, col, 1.0)
    dinv = np.where(deg > 0, 1.0 / np.sqrt(deg), 0.0)
    norm = dinv[row] * dinv[col]
    A = np.zeros((NNODES, NNODES), dtype=np.float64)
    np.add.at(A, (col, row), norm)
    return A


def _build_program():
    import concourse.bass as bass
    import concourse.tile as tile
    from concourse import bacc, mybir

    f32 = mybir.dt.float32
    f16 = mybir.dt.float16

    nc = bacc.Bacc(
        "TRN2", target_bir_lowering=False, debug=False, num_devices=N_CORES
    )
    # host-pretransposed x: [64, 60000] fp16 (channel-major)
    xh = nc.dram_tensor("xh", [64, ROWS_LOC], f16, kind="ExternalInput").ap()
    # [Wcat; Wcat] / 2 so the K=128 duplicated-x contraction is exact: [128, 192]
    wh = nc.dram_tensor("wh", [128, 3 * C], f16, kind="ExternalInput").ap()
    # three block-diagonal graph stationaries kron(I5, A_k^T): [3, 125, 125]
    ah = nc.dram_tensor("ah", [3, TILE, TILE], f16, kind="ExternalInput").ap()
    # permuted output: dev[p, m, c, t] = out for btn tile 8m+t, row p, chan c
    dev = nc.dram_tensor(
        "dev", [TILE, NTILES // 8, C, 8], f16, kind="ExternalOutput"
    ).ap()

    DEPTH = 1  # software-pipeline distance, in 4-tile groups

    with tile.TileContext(nc) as tc:
        with (
            tc.tile_pool(name="const", bufs=1) as const_pool,
            tc.tile_pool(name="ysb", bufs=4) as ysb_pool,
            tc.tile_pool(name="ostg", bufs=2) as ostg_pool,
            tc.tile_pool(name="ring", bufs=1, space="PSUM") as ring_pool,
        ):
            # the whole of PSUM as one 8-bank ring
            big = ring_pool.tile([128, 8, BANKC], f32, tag="ring", name="ring")

            # constants on the scalar HWDGE queue
            wsb = const_pool.tile([128, 3 * C], f16, tag="wcat")
            nc.scalar.dma_start(wsb[:], wh[:])
            asb = []
            for k in range(3):
                t = const_pool.tile([TILE, TILE], f16, tag=f"a{k}")
                nc.scalar.dma_start(t[:], ah[k])
                asb.append(t)
            # x streamed in NCHUNK big chunks on the sync (SP) queue, then
            # duplicated to partitions 64-127 on-chip so W-matmuls run with
            # K=128 (K<=64 matmuls stream at half rate on the PE)
            xsb = []
            for ci in range(NCHUNK):
                t = const_pool.tile([128, CHW], f16, tag=f"x{ci}")
                nc.sync.dma_start(t[0:64, :], xh[:, ci * CHW : (ci + 1) * CHW])
                nc.scalar.dma_start(t[64:128, :], t[0:64, :])
                xsb.append(t)

            def xchunk(i):
                # lhsT [128, 125] for btn tile i (duplicated channels)
                ci, off = i // (CHW // TILE), (i % (CHW // TILE)) * TILE
                return xsb[ci][0:128, off : off + TILE]

            ysbs = {}

            def copy(n, dst, src):
                # only DVE and ACT can read PSUM
                if n % 2 == 0:
                    nc.scalar.copy(dst, src)
                else:
                    nc.vector.tensor_copy(dst, src)

            ncopies = [0]
            NSG = NTILES // 8  # 60 super-groups of 8 tiles

            OG = 4  # super-groups per out staging buffer / DMA
            ostg = {"t": None}

            def emit_a_mm(m, k):
                # one accumulating A-matmul, F=512, out = full bank 6 or 7
                ysb = ysbs[m]
                ob = 6 + (m % 2)
                nc.tensor.matmul(
                    big[0:TILE, ob, 0:BANKC],
                    asb[k][:],
                    ysb[0:TILE, k, :, :],
                    start=(k == 0), stop=(k == 2),
                )

            def emit_a_out(m):
                ysbs.pop(m)
                ob = 6 + (m % 2)
                go = m % OG
                if go == 0:
                    ostg["t"] = ostg_pool.tile(
                        [TILE, OG, BANKC], f16, tag="ostg", name="ostg"
                    )
                st = ostg["t"]
                copy(ncopies[0], st[0:TILE, go, 0:BANKC], big[0:TILE, ob, 0:BANKC])
                ncopies[0] += 1
                if go == OG - 1:
                    blk = m // OG
                    nc.gpsimd.dma_start(
                        dev[:, OG * blk : OG * (blk + 1), :, :], st[:]
                    )

            def emit_sg(m):
                # 8 W-matmuls into the Y ring (banks 0-5) with the previous
                # super-group's three fat A-matmuls interleaved between the
                # W-pairs, so the PE has dependency-free work while the Y
                # cast-copies drain the ring behind it.
                ysb = None
                if m < NSG:
                    ysb = ysb_pool.tile([TILE, 3, C, 8], f16, tag="y", name="y")
                    ysbs[m] = ysb
                for h in range(4):
                    if m < NSG:
                        for u in range(2):
                            i = 8 * m + 2 * h + u
                            s = i % 6
                            nc.tensor.matmul(
                                big[0:TILE, s, 0 : 3 * C],
                                xchunk(i), wsb[:],
                                start=True, stop=True,
                            )
                    if m >= DEPTH and h < 3:
                        emit_a_mm(m - DEPTH, h)
                    if m < NSG:
                        s = (8 * m + 2 * h) % 6
                        # transpose (t', k, c) -> ysb[k, c, t] in the AP walk
                        dst = ysb[0:TILE, :, :, 2 * h : 2 * h + 2].rearrange(
                            "p k c t -> p t k c"
                        )
                        copy(
                            ncopies[0],
                            dst,
                            big[0:TILE, s : s + 2, 0 : 3 * C],
                        )
                        ncopies[0] += 1
                if m >= DEPTH:
                    emit_a_out(m - DEPTH)

            for m in range(NSG + DEPTH):
                emit_sg(m)

    nc.compile()
    return nc


def kernel(x, edge_index, W1, W2, W3, b1, b2, b3):
    from concourse.bass_utils import run_bass_kernel_spmd

    x = np.asarray(x, dtype=np.float32)
    edge_index = np.asarray(edge_index)
    Ws = [np.asarray(W, dtype=np.float64) for W in (W1, W2, W3)]
    bs = [np.asarray(b, dtype=np.float64) for b in (b1, b2, b3)]

    # host-side operator prep
    Wcat = np.concatenate(Ws, axis=1)  # [64, 192]
    wh = (np.vstack([Wcat, Wcat]) / 2.0).astype(np.float16)  # [128, 192]
    ah = np.zeros((3, TILE, TILE), dtype=np.float16)
    for k in range(3):
        Ak = _dense_adj(edge_index[k])
        blk = Ak.T.astype(np.float16)
        for g in range(5):
            ah[k, g * NNODES : (g + 1) * NNODES, g * NNODES : (g + 1) * NNODES] = blk
    bias = np.zeros(C, dtype=np.float64)
    for b in bs:
        bias += b

    if "nc" not in _PROGRAM_CACHE:
        _PROGRAM_CACHE["nc"] = _build_program()
    nc = _PROGRAM_CACHE["nc"]

    # per-core host-transposed x: [64, 60000] -> [128, 30000]
    x16 = x.reshape(N_CORES, ROWS_LOC, C).astype(np.float16)
    in_maps = []
    for i in range(N_CORES):
        xT = np.ascontiguousarray(x16[i].T)  # [64, 60000]
        in_maps.append({"xh": xT, "wh": wh, "ah": ah})

    res = run_bass_kernel_spmd(nc, in_maps, list(range(N_CORES)), **_RUN_KW)
    _PROGRAM_CACHE["last_result"] = res

    # un-permute: dev[p, m, c, t] = tile (8m+t), row p = 25g+n2, chan c
    outs = []
    for i in range(N_CORES):
        d = res.results[i]["dev"].astype(np.float32)  # [125, 60, 64, 8]
        arr = d.transpose(1, 3, 0, 2).reshape(ROWS_LOC, C)
        outs.append(arr.reshape(BT_LOC, NNODES, C))
    out = np.stack(outs, axis=0)
    if np.any(bias):
        out += bias.astype(np.float32)[None, None, None, :]
    return np.ascontiguousarray(
        out.reshape(B, T, NNODES, C)
    )


# revision 21
# speedup vs baseline: 1.4666x; 1.4666x over previous
"""Trainium2 Bass kernel for a 3-branch GCN layer (sum of three GCNConvs).

Math: out[b,t] = sum_k A_k @ (x[b,t] @ W_k) + b_k, with A_k the normalized
adjacency (self loops) of the k-th tiny 25-node graph shared across (B,T).

Instead of the dense [1600x1600] kron operator (one big GEMM, ~395k PE
row-cycles/core), factor into two chained PE stages with NO on-chip
transposes (host pre-transposes x, which is free):

  stage W:  Y[btn, (k,c)] = X[btn, c'] @ [W1|W2|W3]      (K=64, F=192)
  stage A:  out[btn, c]   = sum_k kron(I5, A_k^T) @ Y_k  (K=125, F=64 x3)

Tiles are 125 rows = 5 (b,t) groups x 25 nodes, so the graph contraction
is a fixed 125x125 block-diagonal stationary per branch (~184k PE
row-cycles/core total). PSUM is managed as one 8-bank ring; each bank
holds one tile's Y accumulation region and its out region (psum
accumulation state is bank-granular, so never two accumulation groups
per bank). Y is cast fp32->fp16 by batched pair-copies spread over
DVE/ACT/GPSIMD; out is DMA'd directly from PSUM as fp32.

Data-parallel over batch: 8 batches (2400 bt rows) per core x 8 cores.
Bias is added on the host (typically zero; np.any fast-path).
"""

import sys

import numpy as np

if "/opt/trn_rl_repo" not in sys.path:
    sys.path.insert(0, "/opt/trn_rl_repo")

B, T, NNODES, C = 64, 300, 25, 64
N_CORES = 8
BT_LOC = (B // N_CORES) * T          # 2400 (b,t) rows per core
ROWS_LOC = BT_LOC * NNODES           # 60000 btn rows per core
TILE = 125                           # 5 bt-groups x 25 nodes
NTILES = ROWS_LOC // TILE            # 480
NGRP = NTILES // 4                   # 120 groups of 4 tiles
NCHUNK = 8                           # x input DMA chunks
HALF = NTILES // 2                   # tiles per partition-half (240)
CHW = ROWS_LOC // NCHUNK             # x chunk width in elements (7500)
BANKC = 512                          # fp32 elems per psum bank partition-row
OOFF = 256                           # out region offset within a bank

_PROGRAM_CACHE = {}
# extra kwargs for run_bass_kernel_spmd (test harness sets trace=True here)
_RUN_KW = {}


def _dense_adj(edge_index_k: np.ndarray) -> np.ndarray:
    """PyG GCNConv normalized dense adjacency A[dst, src] (float64)."""
    row = edge_index_k[0].astype(np.int64)
    col = edge_index_k[1].astype(np.int64)
    loop = np.arange(NNODES, dtype=np.int64)
    row = np.concatenate([row, loop])
    col = np.concatenate([col, loop])
    deg = np.zeros(NNODES, dtype=np.float64)
    np.add.at(deg, col, 1.0)
    dinv = np.where(deg > 0, 1.0 / np.sqrt(deg), 0.0)
    norm = dinv[row] * dinv[col]
    A = np.zeros((NNODES, NNODES), dtype=np.float64)
    np.add.at(A, (col, row), norm)
    return A


def _build_program():
    import concourse.bass as bass
    import concourse.tile as tile
    from concourse import bacc, mybir

    f32 = mybir.dt.float32
    f16 = mybir.dt.float16

    nc = bacc.Bacc(
        "TRN2", target_bir_lowering=False, debug=False, num_devices=N_CORES
    )
    # host-pretransposed x: [64, 60000] fp16 (channel-major)
    xh = nc.dram_tensor("xh", [64, ROWS_LOC], f16, kind="ExternalInput").ap()
    # [Wcat; Wcat] / 2 so the K=128 duplicated-x contraction is exact: [128, 192]
    wh = nc.dram_tensor("wh", [128, 3 * C], f16, kind="ExternalInput").ap()
    # three zero-padded block-diagonal stationaries kron(I5, A_k^T): [3,128,128]
    ah = nc.dram_tensor("ah", [3, 128, 128], f16, kind="ExternalInput").ap()
    # permuted output: dev[p, m, t, c] = out for btn tile 8m+t, row p, chan c
    dev = nc.dram_tensor(
        "dev", [TILE, NTILES // 8, 8, C], f16, kind="ExternalOutput"
    ).ap()

    DEPTH = 1  # software-pipeline distance, in 4-tile groups

    with tile.TileContext(nc) as tc:
        with (
            tc.tile_pool(name="const", bufs=1) as const_pool,
            tc.tile_pool(name="ysb", bufs=4) as ysb_pool,
            tc.tile_pool(name="ostg", bufs=2) as ostg_pool,
            tc.tile_pool(name="ring", bufs=1, space="PSUM") as ring_pool,
        ):
            # the whole of PSUM as one 8-bank ring
            big = ring_pool.tile([128, 8, BANKC], f32, tag="ring", name="ring")

            # constants on the scalar HWDGE queue
            wsb = const_pool.tile([128, 3 * C], f16, tag="wcat")
            nc.scalar.dma_start(wsb[:], wh[:])
            asb = []
            for k in range(3):
                t = const_pool.tile([128, 128], f16, tag=f"a{k}")
                nc.scalar.dma_start(t[:], ah[k])
                asb.append(t)
            
            # x streamed in NCHUNK big chunks on the sync (SP) queue, then
            # duplicated to partitions 64-127 on-chip so W-matmuls run with
            # K=128 (K<=64 matmuls stream at half rate on the PE)
            xsb = []
            for ci in range(NCHUNK):
                t = const_pool.tile([128, CHW], f16, tag=f"x{ci}")
                nc.sync.dma_start(t[0:64, :], xh[:, ci * CHW : (ci + 1) * CHW])
                nc.scalar.dma_start(t[64:128, :], t[0:64, :])
                xsb.append(t)

            TPC = CHW // TILE  # tiles per chunk (60)

            def xchunk(i):
                # lhsT [128, 125] for btn tile i (duplicated channels)
                ci, off = i // TPC, (i % TPC) * TILE
                return xsb[ci][0:128, off : off + TILE]

            ysbs = {}

            def copy(n, dst, src):
                # only DVE and ACT can read PSUM
                if n % 2 == 0:
                    nc.scalar.copy(dst, src)
                else:
                    nc.vector.tensor_copy(dst, src)

            ncopies = [0]
            NSG = NTILES // 8  # 60 super-groups of 8 tiles

            OG = 4  # super-groups per out staging buffer / DMA
            ostg = {"t": None}

            def emit_a_mm(m, k):
                # one accumulating A-matmul, F=512, out = full bank 6 or 7
                ysb = ysbs[m]
                ob = 6 + (m % 2)
                nc.tensor.matmul(
                    big[0:TILE, ob, 0:BANKC],
                    asb[k][0:TILE, 0:TILE],
                    ysb[0:TILE, 0:8, k * C : (k + 1) * C],
                    start=(k == 0), stop=(k == 2),
                )

            def emit_a_out(m):
                ysbs.pop(m)
                ob = 6 + (m % 2)
                go = m % OG
                if go == 0:
                    ostg["t"] = ostg_pool.tile(
                        [TILE, OG, BANKC], f16, tag="ostg", name="ostg"
                    )
                st = ostg["t"]
                copy(ncopies[0], st[0:TILE, go, 0:BANKC], big[0:TILE, ob, 0:BANKC])
                ncopies[0] += 1
                if go == OG - 1:
                    blk = m // OG
                    nc.gpsimd.dma_start(
                        dev[:, OG * blk : OG * (blk + 1), :, :], st[:]
                    )

            def emit_w_mm(i):
                s = i % 6
                nc.tensor.matmul(
                    big[0:TILE, s, 0 : 3 * C],
                    xchunk(i), wsb[:],
                    start=True, stop=True,
                )

            def emit_y_copy(m, h, ysb):
                s = (8 * m + 2 * h) % 6
                copy(
                    ncopies[0],
                    ysb[0:TILE, 2 * h : 2 * h + 2, :],
                    big[0:TILE, s : s + 2, 0 : 3 * C],
                )
                ncopies[0] += 1

            def emit_sg(m):
                # W-pairs with the previous super-group's fat A-matmuls
                # interleaved between them (v4 ordering)
                ysb = None
                if m < NSG:
                    ysb = ysb_pool.tile([128, 8, 3 * C], f16, tag="y", name="y")
                    ysbs[m] = ysb
                for h in range(4):
                    if m < NSG:
                        emit_w_mm(8 * m + 2 * h)
                        emit_w_mm(8 * m + 2 * h + 1)
                    if m >= DEPTH and h < 3:
                        emit_a_mm(m - DEPTH, h)
                    if m < NSG:
                        emit_y_copy(m, h, ysb)
                if m >= DEPTH:
                    emit_a_out(m - DEPTH)

            for m in range(NSG + DEPTH):
                emit_sg(m)

    nc.compile()
    return nc


def kernel(x, edge_index, W1, W2, W3, b1, b2, b3):
    from concourse.bass_utils import run_bass_kernel_spmd

    x = np.asarray(x, dtype=np.float32)
    edge_index = np.asarray(edge_index)
    Ws = [np.asarray(W, dtype=np.float64) for W in (W1, W2, W3)]
    bs = [np.asarray(b, dtype=np.float64) for b in (b1, b2, b3)]

    # host-side operator prep
    Wcat = np.concatenate(Ws, axis=1)  # [64, 192]
    wh = (np.vstack([Wcat, Wcat]) / 2.0).astype(np.float16)  # [128, 192]
    ah = np.zeros((3, 128, 128), dtype=np.float16)
    for k in range(3):
        Ak = _dense_adj(edge_index[k])
        blk = Ak.T.astype(np.float16)
        for g in range(5):
            ah[k, g * NNODES : (g + 1) * NNODES, g * NNODES : (g + 1) * NNODES] = blk
    bias = np.zeros(C, dtype=np.float64)
    for b in bs:
        bias += b

    if "nc" not in _PROGRAM_CACHE:
        _PROGRAM_CACHE["nc"] = _build_program()
    nc = _PROGRAM_CACHE["nc"]

    # per-core host-transposed x: [64, 60000] -> [128, 30000]
    x16 = x.reshape(N_CORES, ROWS_LOC, C).astype(np.float16)
    in_maps = []
    for i in range(N_CORES):
        xT = np.ascontiguousarray(x16[i].T)  # [64, 60000]
        in_maps.append({"xh": xT, "wh": wh, "ah": ah})

    res = run_bass_kernel_spmd(nc, in_maps, list(range(N_CORES)), **_RUN_KW)
    _PROGRAM_CACHE["last_result"] = res

    # un-permute: dev[p, m, t, c] = tile (8m+t), row p = 25g+n2, chan c
    outs = []
    for i in range(N_CORES):
        d = res.results[i]["dev"].astype(np.float32)  # [125, 60, 8, 64]
        arr = d.transpose(1, 2, 0, 3).reshape(ROWS_LOC, C)
        outs.append(arr.reshape(BT_LOC, NNODES, C))
    out = np.stack(outs, axis=0)
    if np.any(bias):
        out += bias.astype(np.float32)[None, None, None, :]
    return np.ascontiguousarray(
        out.reshape(B, T, NNODES, C)
    )


# revision 22
# speedup vs baseline: 1.7177x; 1.1712x over previous
"""Trainium2 Bass kernel for a 3-branch GCN layer (sum of three GCNConvs).

Math: out[b,t] = sum_k A_k @ (x[b,t] @ W_k) + b_k, with A_k the normalized
adjacency (self loops) of the k-th tiny 25-node graph shared across (B,T).

Instead of the dense [1600x1600] kron operator (one big GEMM, ~395k PE
row-cycles/core), factor into two chained PE stages with NO on-chip
transposes (host pre-transposes x, which is free):

  stage W:  Y[btn, (k,c)] = X[btn, c'] @ [W1|W2|W3]      (K=64, F=192)
  stage A:  out[btn, c]   = sum_k kron(I5, A_k^T) @ Y_k  (K=125, F=64 x3)

Tiles are 125 rows = 5 (b,t) groups x 25 nodes, so the graph contraction
is a fixed 125x125 block-diagonal stationary per branch (~184k PE
row-cycles/core total). PSUM is managed as one 8-bank ring; each bank
holds one tile's Y accumulation region and its out region (psum
accumulation state is bank-granular, so never two accumulation groups
per bank). Y is cast fp32->fp16 by batched pair-copies spread over
DVE/ACT/GPSIMD; out is DMA'd directly from PSUM as fp32.

Data-parallel over batch: 8 batches (2400 bt rows) per core x 8 cores.
Bias is added on the host (typically zero; np.any fast-path).
"""

import sys

import numpy as np

if "/opt/trn_rl_repo" not in sys.path:
    sys.path.insert(0, "/opt/trn_rl_repo")

B, T, NNODES, C = 64, 300, 25, 64
N_CORES = 8
BT_LOC = (B // N_CORES) * T          # 2400 (b,t) rows per core
ROWS_LOC = BT_LOC * NNODES           # 60000 btn rows per core
TILE = 125                           # 5 bt-groups x 25 nodes
NTILES = ROWS_LOC // TILE            # 480
NGRP = NTILES // 4                   # 120 groups of 4 tiles
NCHUNK = 8                           # x input DMA chunks
HALF = NTILES // 2                   # tiles per partition-half (240)
CHW = NTILES * 128 // NCHUNK         # x chunk width in elements (7680)
BANKC = 512                          # fp32 elems per psum bank partition-row
OOFF = 256                           # out region offset within a bank

_PROGRAM_CACHE = {}
# extra kwargs for run_bass_kernel_spmd (test harness sets trace=True here)
_RUN_KW = {}


def _dense_adj(edge_index_k: np.ndarray) -> np.ndarray:
    """PyG GCNConv normalized dense adjacency A[dst, src] (float64)."""
    row = edge_index_k[0].astype(np.int64)
    col = edge_index_k[1].astype(np.int64)
    loop = np.arange(NNODES, dtype=np.int64)
    row = np.concatenate([row, loop])
    col = np.concatenate([col, loop])
    deg = np.zeros(NNODES, dtype=np.float64)
    np.add.at(deg, col, 1.0)
    dinv = np.where(deg > 0, 1.0 / np.sqrt(deg), 0.0)
    norm = dinv[row] * dinv[col]
    A = np.zeros((NNODES, NNODES), dtype=np.float64)
    np.add.at(A, (col, row), norm)
    return A


def _build_program():
    import concourse.bass as bass
    import concourse.tile as tile
    from concourse import bacc, mybir

    f32 = mybir.dt.float32
    f16 = mybir.dt.float16

    nc = bacc.Bacc(
        "TRN2", target_bir_lowering=False, debug=False, num_devices=N_CORES
    )
    # host-pretransposed x, 128-col padded tiles: [64, 480*128] fp16
    xh = nc.dram_tensor("xh", [64, NTILES * 128], f16, kind="ExternalInput").ap()
    # [Wcat; Wcat] / 2 so the K=128 duplicated-x contraction is exact: [128, 192]
    wh = nc.dram_tensor("wh", [128, 3 * C], f16, kind="ExternalInput").ap()
    # three zero-padded block-diagonal stationaries kron(I5, A_k^T): [3,128,128]
    ah = nc.dram_tensor("ah", [3, 128, 128], f16, kind="ExternalInput").ap()
    # permuted output: dev[p, m, t, c] = out for btn tile 8m+t, row p, chan c
    dev = nc.dram_tensor(
        "dev", [TILE, NTILES // 8, 8, C], f16, kind="ExternalOutput"
    ).ap()

    DEPTH = 1  # software-pipeline distance, in 4-tile groups

    with tile.TileContext(nc) as tc:
        with (
            tc.tile_pool(name="const", bufs=1) as const_pool,
            tc.tile_pool(name="ysb", bufs=4) as ysb_pool,
            tc.tile_pool(name="ostg", bufs=2) as ostg_pool,
            tc.tile_pool(name="ring", bufs=1, space="PSUM") as ring_pool,
        ):
            # the whole of PSUM as one 8-bank ring
            big = ring_pool.tile([128, 8, BANKC], f32, tag="ring", name="ring")

            # constants on the scalar HWDGE queue
            wsb = const_pool.tile([128, 3 * C], f16, tag="wcat")
            nc.scalar.dma_start(wsb[:], wh[:])
            asb = []
            for k in range(3):
                t = const_pool.tile([128, 128], f16, tag=f"a{k}")
                nc.scalar.dma_start(t[:], ah[k])
                asb.append(t)
            
            # x streamed in NCHUNK big chunks on the sync (SP) queue, then
            # duplicated to partitions 64-127 on-chip so W-matmuls run with
            # K=128 (K<=64 matmuls stream at half rate on the PE)
            xsb = []
            for ci in range(NCHUNK):
                t = const_pool.tile([128, CHW], f16, tag=f"x{ci}")
                nc.sync.dma_start(t[0:64, :], xh[:, ci * CHW : (ci + 1) * CHW])
                nc.scalar.dma_start(t[64:128, :], t[0:64, :])
                xsb.append(t)

            TPC = CHW // 128  # tiles per chunk (60)

            def xchunk(i):
                # lhsT [128, 128] for btn tile i (dup channels, 3 zero pad cols)
                ci, off = i // TPC, (i % TPC) * 128
                return xsb[ci][0:128, off : off + 128]

            ysbs = {}

            def copy(n, dst, src):
                # only DVE and ACT can read PSUM
                if n % 2 == 0:
                    nc.scalar.copy(dst, src)
                else:
                    nc.vector.tensor_copy(dst, src)

            ncopies = [0]
            NSG = NTILES // 8  # 60 super-groups of 8 tiles

            OG = 4  # super-groups per out staging buffer / DMA
            ostg = {"t": None}

            def emit_a_mm(m, k):
                # one accumulating A-matmul, F=512, out = full bank 6 or 7
                ysb = ysbs[m]
                ob = 6 + (m % 2)
                nc.tensor.matmul(
                    big[0:128, ob, 0:BANKC],
                    asb[k][:],
                    ysb[0:128, k, :, :],
                    start=(k == 0), stop=(k == 2),
                )

            def emit_a_out(m):
                ysbs.pop(m)
                ob = 6 + (m % 2)
                go = m % OG
                if go == 0:
                    ostg["t"] = ostg_pool.tile(
                        [TILE, OG, BANKC], f16, tag="ostg", name="ostg"
                    )
                st = ostg["t"]
                copy(ncopies[0], st[0:TILE, go, 0:BANKC], big[0:TILE, ob, 0:BANKC])
                ncopies[0] += 1
                if go == OG - 1:
                    blk = m // OG
                    nc.gpsimd.dma_start(
                        dev[:, OG * blk : OG * (blk + 1), :, :], st[:]
                    )

            def emit_w_mm(i):
                s = i % 6
                nc.tensor.matmul(
                    big[0:128, s, 0 : 3 * C],
                    xchunk(i), wsb[:],
                    start=True, stop=True,
                )

            def emit_y_copy(m, h, ysb):
                s = (8 * m + 2 * h) % 6
                # dst walk order (t', k, c) to match the psum source; 64-elem
                # contiguous runs keep the engine AP walker fast
                dst = ysb[0:128, :, 2 * h : 2 * h + 2, :].rearrange(
                    "p k t c -> p t k c"
                )
                copy(
                    ncopies[0],
                    dst,
                    big[0:128, s : s + 2, 0 : 3 * C],
                )
                ncopies[0] += 1

            def emit_sg(m):
                # W-pairs with the previous super-group's fat A-matmuls
                # interleaved between them (v4 ordering)
                ysb = None
                if m < NSG:
                    ysb = ysb_pool.tile([128, 3, 8, C], f16, tag="y", name="y")
                    ysbs[m] = ysb
                for h in range(4):
                    if m < NSG:
                        emit_w_mm(8 * m + 2 * h)
                        emit_w_mm(8 * m + 2 * h + 1)
                    if m >= DEPTH and h < 3:
                        emit_a_mm(m - DEPTH, h)
                    if m < NSG:
                        emit_y_copy(m, h, ysb)
                if m >= DEPTH:
                    emit_a_out(m - DEPTH)

            for m in range(NSG + DEPTH):
                emit_sg(m)

    nc.compile()
    return nc


def kernel(x, edge_index, W1, W2, W3, b1, b2, b3):
    from concourse.bass_utils import run_bass_kernel_spmd

    x = np.asarray(x, dtype=np.float32)
    edge_index = np.asarray(edge_index)
    Ws = [np.asarray(W, dtype=np.float64) for W in (W1, W2, W3)]
    bs = [np.asarray(b, dtype=np.float64) for b in (b1, b2, b3)]

    # host-side operator prep
    Wcat = np.concatenate(Ws, axis=1)  # [64, 192]
    wh = (np.vstack([Wcat, Wcat]) / 2.0).astype(np.float16)  # [128, 192]
    ah = np.zeros((3, 128, 128), dtype=np.float16)
    for k in range(3):
        Ak = _dense_adj(edge_index[k])
        blk = Ak.T.astype(np.float16)
        for g in range(5):
            ah[k, g * NNODES : (g + 1) * NNODES, g * NNODES : (g + 1) * NNODES] = blk
    bias = np.zeros(C, dtype=np.float64)
    for b in bs:
        bias += b

    if "nc" not in _PROGRAM_CACHE:
        _PROGRAM_CACHE["nc"] = _build_program()
    nc = _PROGRAM_CACHE["nc"]

    # per-core host-transposed x: [64, 60000] -> [128, 30000]
    x16 = x.reshape(N_CORES, ROWS_LOC, C).astype(np.float16)
    in_maps = []
    for i in range(N_CORES):
        xT = x16[i].T  # [64, 60000]
        xp = np.zeros((C, NTILES, 128), dtype=np.float16)
        xp[:, :, :TILE] = xT.reshape(C, NTILES, TILE)
        in_maps.append({"xh": xp.reshape(C, NTILES * 128), "wh": wh, "ah": ah})

    res = run_bass_kernel_spmd(nc, in_maps, list(range(N_CORES)), **_RUN_KW)
    _PROGRAM_CACHE["last_result"] = res

    # un-permute: dev[p, m, t, c] = tile (8m+t), row p = 25g+n2, chan c
    outs = []
    for i in range(N_CORES):
        d = res.results[i]["dev"].astype(np.float32)  # [125, 60, 8, 64]
        arr = d.transpose(1, 2, 0, 3).reshape(ROWS_LOC, C)
        outs.append(arr.reshape(BT_LOC, NNODES, C))
    out = np.stack(outs, axis=0)
    if np.any(bias):
        out += bias.astype(np.float32)[None, None, None, :]
    return np.ascontiguousarray(
        out.reshape(B, T, NNODES, C)
    )


# revision 23
# speedup vs baseline: 2.6094x; 1.5192x over previous
"""Trainium2 Bass kernel for a 3-branch GCN layer (sum of three GCNConvs).

Math: out[b,t] = sum_k A_k @ (x[b,t] @ W_k) + b_k, with A_k the normalized
adjacency (self loops) of the k-th tiny 25-node graph shared across (B,T).

Instead of the dense [1600x1600] kron operator (one big GEMM, ~395k PE
row-cycles/core), factor into two chained PE stages with NO on-chip
transposes (host pre-transposes x, which is free):

  stage W:  Y[btn, (k,c)] = X[btn, c'] @ [W1|W2|W3]      (K=64, F=192)
  stage A:  out[btn, c]   = sum_k kron(I5, A_k^T) @ Y_k  (K=125, F=64 x3)

Tiles are 125 rows = 5 (b,t) groups x 25 nodes, so the graph contraction
is a fixed 125x125 block-diagonal stationary per branch (~184k PE
row-cycles/core total). PSUM is managed as one 8-bank ring; each bank
holds one tile's Y accumulation region and its out region (psum
accumulation state is bank-granular, so never two accumulation groups
per bank). Y is cast fp32->fp16 by batched pair-copies spread over
DVE/ACT/GPSIMD; out is DMA'd directly from PSUM as fp32.

Data-parallel over batch: 8 batches (2400 bt rows) per core x 8 cores.
Bias is added on the host (typically zero; np.any fast-path).
"""

import sys

import numpy as np

if "/opt/trn_rl_repo" not in sys.path:
    sys.path.insert(0, "/opt/trn_rl_repo")

B, T, NNODES, C = 64, 300, 25, 64
N_CORES = 8
BT_LOC = (B // N_CORES) * T          # 2400 (b,t) rows per core
ROWS_LOC = BT_LOC * NNODES           # 60000 btn rows per core
TILE = 125                           # 5 bt-groups x 25 nodes
NTILES = ROWS_LOC // TILE            # 480
NGRP = NTILES // 4                   # 120 groups of 4 tiles
NCHUNK = 8                           # x input DMA chunks
HALF = NTILES // 2                   # tiles per partition-half (240)
CHW = NTILES // 2 * 128 // NCHUNK    # x chunk width in elements (3840)
BANKC = 512                          # fp32 elems per psum bank partition-row
OOFF = 256                           # out region offset within a bank

_PROGRAM_CACHE = {}
# extra kwargs for run_bass_kernel_spmd (test harness sets trace=True here)
_RUN_KW = {}


def _dense_adj(edge_index_k: np.ndarray) -> np.ndarray:
    """PyG GCNConv normalized dense adjacency A[dst, src] (float64)."""
    row = edge_index_k[0].astype(np.int64)
    col = edge_index_k[1].astype(np.int64)
    loop = np.arange(NNODES, dtype=np.int64)
    row = np.concatenate([row, loop])
    col = np.concatenate([col, loop])
    deg = np.zeros(NNODES, dtype=np.float64)
    np.add.at(deg, col, 1.0)
    dinv = np.where(deg > 0, 1.0 / np.sqrt(deg), 0.0)
    norm = dinv[row] * dinv[col]
    A = np.zeros((NNODES, NNODES), dtype=np.float64)
    np.add.at(A, (col, row), norm)
    return A


def _build_program():
    import concourse.bass as bass
    import concourse.tile as tile
    from concourse import bacc, mybir

    f32 = mybir.dt.float32
    f16 = mybir.dt.float16

    nc = bacc.Bacc(
        "TRN2", target_bir_lowering=False, debug=False, num_devices=N_CORES
    )
    # host-pretransposed x, tile-pair stacked: [128, 240*128] fp16; rows
    # 0-63 = channels of even tile, 64-127 = channels of odd tile
    xh = nc.dram_tensor(
        "xh", [128, NTILES // 2 * 128], f16, kind="ExternalInput"
    ).ap()
    # block-diagonal [[Wcat, 0], [0, Wcat]] for pair-fused W-matmuls
    wh = nc.dram_tensor("wh", [128, 6 * C], f16, kind="ExternalInput").ap()
    # three zero-padded block-diagonal stationaries kron(I5, A_k^T): [3,128,128]
    ah = nc.dram_tensor("ah", [3, 128, 128], f16, kind="ExternalInput").ap()
    # permuted output: dev[p, m, t, c] = out for btn tile 8m+t, row p, chan c
    dev = nc.dram_tensor(
        "dev", [TILE, NTILES // 8, 8, C], f16, kind="ExternalOutput"
    ).ap()

    DEPTH = 1  # software-pipeline distance, in 4-tile groups

    with tile.TileContext(nc) as tc:
        with (
            tc.tile_pool(name="const", bufs=1) as const_pool,
            tc.tile_pool(name="ysb", bufs=4) as ysb_pool,
            tc.tile_pool(name="ostg", bufs=2) as ostg_pool,
            tc.tile_pool(name="ring", bufs=1, space="PSUM") as ring_pool,
        ):
            # the whole of PSUM as one 8-bank ring
            big = ring_pool.tile([128, 8, BANKC], f32, tag="ring", name="ring")

            # constants on the scalar HWDGE queue
            wsb = const_pool.tile([128, 6 * C], f16, tag="wcat")
            nc.scalar.dma_start(wsb[:], wh[:])
            asb = []
            for k in range(3):
                t = const_pool.tile([128, 128], f16, tag=f"a{k}")
                nc.scalar.dma_start(t[:], ah[k])
                asb.append(t)
            
            # x streamed in NCHUNK big chunks on the sync (SP) queue
            xsb = []
            for ci in range(NCHUNK):
                t = const_pool.tile([128, CHW], f16, tag=f"x{ci}")
                nc.sync.dma_start(t[:], xh[:, ci * CHW : (ci + 1) * CHW])
                xsb.append(t)

            PPC = CHW // 128  # tile-pairs per chunk (30)

            def xpair(p):
                # lhsT [128, 128] for tile pair p = tiles (2p, 2p+1)
                ci, off = p // PPC, (p % PPC) * 128
                return xsb[ci][0:128, off : off + 128]

            ysbs = {}

            def copy(n, dst, src):
                # only DVE and ACT can read PSUM
                if n % 2 == 0:
                    nc.scalar.copy(dst, src)
                else:
                    nc.vector.tensor_copy(dst, src)

            ncopies = [0]
            NSG = NTILES // 8  # 60 super-groups of 8 tiles

            OG = 4  # super-groups per out staging buffer / DMA
            ostg = {"t": None}

            def emit_a_mm(m, k):
                # one accumulating A-matmul, F=512, out = full bank 6 or 7
                ysb = ysbs[m]
                ob = 6 + (m % 2)
                nc.tensor.matmul(
                    big[0:128, ob, 0:BANKC],
                    asb[k][:],
                    ysb[0:128, k, :, :],
                    start=(k == 0), stop=(k == 2),
                )

            def emit_a_out(m):
                ysbs.pop(m)
                ob = 6 + (m % 2)
                go = m % OG
                if go == 0:
                    ostg["t"] = ostg_pool.tile(
                        [TILE, OG, BANKC], f16, tag="ostg", name="ostg"
                    )
                st = ostg["t"]
                copy(ncopies[0], st[0:TILE, go, 0:BANKC], big[0:TILE, ob, 0:BANKC])
                ncopies[0] += 1
                if go == OG - 1:
                    blk = m // OG
                    nc.gpsimd.dma_start(
                        dev[:, OG * blk : OG * (blk + 1), :, :], st[:]
                    )

            def emit_w_mm(p):
                s = p % 6
                nc.tensor.matmul(
                    big[0:128, s, 0 : 6 * C],
                    xpair(p), wsb[:],
                    start=True, stop=True,
                )

            def emit_y_copy(m, h, ysb):
                s = (4 * m + h) % 6
                # dst walk order (t', k, c) to match the psum source; 64-elem
                # contiguous runs keep the engine AP walker fast
                dst = ysb[0:128, :, 2 * h : 2 * h + 2, :].rearrange(
                    "p k t c -> p t k c"
                )
                copy(
                    ncopies[0],
                    dst,
                    big[0:128, s, 0 : 6 * C],
                )
                ncopies[0] += 1

            def emit_sg(m):
                # W-pairs with the previous super-group's fat A-matmuls
                # interleaved between them (v4 ordering)
                ysb = None
                if m < NSG:
                    ysb = ysb_pool.tile([128, 3, 8, C], f16, tag="y", name="y")
                    ysbs[m] = ysb
                for h in range(4):
                    if m < NSG:
                        emit_w_mm(4 * m + h)
                    if m >= DEPTH and h < 3:
                        emit_a_mm(m - DEPTH, h)
                    if m < NSG:
                        emit_y_copy(m, h, ysb)
                if m >= DEPTH:
                    emit_a_out(m - DEPTH)

            for m in range(NSG + DEPTH):
                emit_sg(m)

    nc.compile()
    return nc


def kernel(x, edge_index, W1, W2, W3, b1, b2, b3):
    from concourse.bass_utils import run_bass_kernel_spmd

    x = np.asarray(x, dtype=np.float32)
    edge_index = np.asarray(edge_index)
    Ws = [np.asarray(W, dtype=np.float64) for W in (W1, W2, W3)]
    bs = [np.asarray(b, dtype=np.float64) for b in (b1, b2, b3)]

    # host-side operator prep
    Wcat = np.concatenate(Ws, axis=1)  # [64, 192]
    wh = np.zeros((128, 6 * C), dtype=np.float16)
    wh[0:64, 0 : 3 * C] = Wcat.astype(np.float16)
    wh[64:128, 3 * C : 6 * C] = Wcat.astype(np.float16)
    ah = np.zeros((3, 128, 128), dtype=np.float16)
    for k in range(3):
        Ak = _dense_adj(edge_index[k])
        blk = Ak.T.astype(np.float16)
        for g in range(5):
            ah[k, g * NNODES : (g + 1) * NNODES, g * NNODES : (g + 1) * NNODES] = blk
    bias = np.zeros(C, dtype=np.float64)
    for b in bs:
        bias += b

    if "nc" not in _PROGRAM_CACHE:
        _PROGRAM_CACHE["nc"] = _build_program()
    nc = _PROGRAM_CACHE["nc"]

    # per-core host-transposed x: [64, 60000] -> [128, 30000]
    x16 = x.reshape(N_CORES, ROWS_LOC, C).astype(np.float16)
    in_maps = []
    for i in range(N_CORES):
        xT = x16[i].T.reshape(C, NTILES // 2, 2, TILE)  # [64, 240, 2, 125]
        xp = np.zeros((128, NTILES // 2, 128), dtype=np.float16)
        xp[0:64, :, :TILE] = xT[:, :, 0]
        xp[64:128, :, :TILE] = xT[:, :, 1]
        in_maps.append(
            {"xh": xp.reshape(128, NTILES // 2 * 128), "wh": wh, "ah": ah}
        )

    res = run_bass_kernel_spmd(nc, in_maps, list(range(N_CORES)), **_RUN_KW)
    _PROGRAM_CACHE["last_result"] = res

    # un-permute: dev[p, m, t, c] = tile (8m+t), row p = 25g+n2, chan c
    outs = []
    for i in range(N_CORES):
        d = res.results[i]["dev"].astype(np.float32)  # [125, 60, 8, 64]
        arr = d.transpose(1, 2, 0, 3).reshape(ROWS_LOC, C)
        outs.append(arr.reshape(BT_LOC, NNODES, C))
    out = np.stack(outs, axis=0)
    if np.any(bias):
        out += bias.astype(np.float32)[None, None, None, :]
    return np.ascontiguousarray(
        out.reshape(B, T, NNODES, C)
    )
